# revision 53
# baseline (speedup 1.0000x reference)
"""Multi-head causal self-attention (B=2, N=4096, C=512, H=8, D=64) on 8 TRN2 cores.

Sharding: core = b*4 + g  (b = batch 0..1, g = head-group 0..3, 2 heads each).

v3 restructure vs v2 baseline (156.6us -> 138.2us cost-model time):
- QK matmuls run in fp8e4 DoubleRow (0.5 cyc/row): K is single-fp8
  duplicated into the DR pair via a stride-0 broadcast AP (stationary),
  Q is an (hi, lo) compensated fp8 pair (moving). PE QK cycles halve;
  adds ~1% output error from the 3.6%-rms K quantization (total
  1.11e-2 vs the 2e-2 gate).
- exp work is split ACT/DVE by a greedy build-time load balancer that
  also routes the PSUM-evacuation copies (q_hi/k8/v, y staging).
- QKV-phase matmuls get their own 1-bank PSUM slot (shared with the
  projection psY) so the S-unit 3-slot rotation is not coupled to them.
- diag-mask multiplies move to the Pool engine (SBUF-only, idle).
- softmax normalization is ONE tensor_tensor per (qt, h) using a
  stride-0 broadcast of the per-(q,qc) reciprocal.
- v ones-columns are memset directly (strided), not the whole tile.
- x and qkv weights are host-retiled partition-major so every DMA
  descriptor is a full 1-2KB partition line; wq rides the SP queue
  ahead of x (it gates the first matmul).
- drain: the last stream emits its diag unit first and runs tight AV
  deferrals; epilogues are guarded to always flush after their AVs
  (an earlier variant silently dropped the last key blocks there).

All SEQ queues dispatch strictly in order, so emission order is the
schedule; the K_* env knobs (defaults = tuned values) control the
deferral lags.
"""

import os

import numpy as np
import ml_dtypes

_CACHE: dict = {}
LAST_RESULTS = None

B, C = 2, 512
H, D = 8, 64
N = 4096
NQT = 8          # q tiles of 512
NKB = 32         # key blocks of 128
QT = 512
KB = 128

# Schraudolph bf16-exp constants (i16 = rint(A*s + Bc), RNE convert verified)
SCH_A = 2.0**7 / float(np.log(2.0))
SCH_B = 127.0 * 2.0**7 - 0.0573 * 2.0**7
# QK now runs unscaled fp8 (64x on both q and k): logits = psS / 4096
QK_SCALE = 1.0 / 4096.0


class Balancer:
    """Greedy ACT/DVE load balancing at build time (ns estimates)."""

    def __init__(self, act_bias=0.0):
        self.act = act_bias
        self.dve = 0.0

    def pick(self, act_cost, dve_cost):
        if self.act + act_cost <= self.dve + dve_cost:
            self.act += act_cost
            return "act"
        self.dve += dve_cost
        return "dve"

    def pick_exp(self, act_cost, dve_cost, idx, mode):
        if mode == 1:
            # strict alternation for exp units; totals still tracked
            if idx % 2 == 0:
                self.act += act_cost
                return "act"
            self.dve += dve_cost
            return "dve"
        return self.pick(act_cost, dve_cost)

    def add(self, eng, cost):
        if eng == "act":
            self.act += cost
        else:
            self.dve += cost


def _build():
    import concourse.bass as bass
    import concourse.bacc as bacc
    import concourse.mybir as mybir
    import concourse.tile as tile

    dt = mybir.dt
    bf = dt.bfloat16
    f32 = dt.float32
    i16 = dt.int16
    Exp = mybir.ActivationFunctionType.Exp
    Copy = mybir.ActivationFunctionType.Copy
    Alu = mybir.AluOpType

    f8 = dt.float8e4
    DR = mybir.MatmulPerfMode.DoubleRow
    EPI_LAG = int(os.environ.get("K_EPI_LAG", "3"))
    AVD_LAG = int(os.environ.get("K_AVD_LAG", "3"))
    AVP_LAG = int(os.environ.get("K_AVP_LAG", "4"))
    PROJ_LAG = int(os.environ.get("K_PROJ_LAG", "8"))
    HEAD_SPLIT = os.environ.get("K_HEAD_SPLIT", "0") == "1"
    TAVP_LAG = int(os.environ.get("K_TAVP_LAG", "4"))
    Y_ALL_SYNC = os.environ.get("K_Y_ALL_SYNC", "0") == "1"
    PATTERN = int(os.environ.get("K_PATTERN", "0"))
    TICK_MODE = int(os.environ.get("K_TICK_MODE", "0"))
    ILV = os.environ.get("K_ILV", "1") == "1"
    ILV_OFS = int(os.environ.get("K_ILV_OFS", "4"))
    ILV_BS = os.environ.get("K_ILV_BS", "0") == "1"
    ILV5 = os.environ.get("K_ILV5", "0") == "1"
    ILV_LO = NQT - 3 if ILV5 else NQT - 2
    POOL_Y = os.environ.get("K_POOL_Y", "0") == "1"
    POOL_QLO = os.environ.get("K_POOL_QLO", "0") == "1"
    MASK_DVE = os.environ.get("K_MASK_DVE", "0") == "1"
    YDMA_POOL = os.environ.get("K_YDMA_POOL", "0") == "1"
    POOL_PA = os.environ.get("K_POOL_PA", "0") == "1"
    Q_SINGLE = os.environ.get("K_QSINGLE", "0") == "1"
    # ordering guards: every AV must flush no later than its stream's
    # epilogue (equal dues resolve by insertion order = AV first)
    assert TAVP_LAG >= 1
    assert AVD_LAG <= EPI_LAG + 1 and AVP_LAG <= EPI_LAG + 2
    PF_BUFS = int(os.environ.get("K_PF_BUFS", "10"))
    TAIL_REORDER = os.environ.get("K_TAIL_REORDER", "1") == "1"
    nc = bacc.Bacc("TRN2", target_bir_lowering=False)
    xth = nc.dram_tensor("xth", [128, NQT, 4, QT], f8, kind="ExternalInput")
    xtl = nc.dram_tensor("xtl", [128, NQT, 4, QT], f8, kind="ExternalInput")
    wq8 = nc.dram_tensor("wq8", [128, 2, 4, 128], f8, kind="ExternalInput")
    wk8 = nc.dram_tensor("wk8", [128, 2, 4, 128], f8, kind="ExternalInput")
    wv8 = nc.dram_tensor("wv8", [128, 2, 4, 128], f8, kind="ExternalInput")
    wp = nc.dram_tensor("wp", [128, C], bf, kind="ExternalInput")
    tri = nc.dram_tensor("tri", [128, 128], bf, kind="ExternalInput")
    dmask = nc.dram_tensor("dmask", [128, 640], bf, kind="ExternalInput")
    YBF = os.environ.get("K_YBF16", "1") == "1"
    yt = nc.dram_tensor("yt", [C, N], bf if YBF else f32, kind="ExternalOutput")

    bal = Balancer(act_bias=float(os.environ.get("K_ACT_BIAS", "0")))
    # per-op engine-time estimates (ns) used only for balancing
    A_EXP = {1024: 1038.0, 512: 612.0, 768: 825.0}
    D_EXP = {1024: 1192.0, 512: 658.0, 768: 925.0}
    A_COPY = 570.0
    D_COPY = 658.0

    with tile.TileContext(nc) as tc:
        with (
            tc.tile_pool(name="persist", bufs=1) as pp,
            tc.tile_pool(name="pf", bufs=PF_BUFS) as pf_pool,      # P tiles (pairs)
            tc.tile_pool(name="pd", bufs=int(os.environ.get("K_PD_BUFS", "8"))) as pd_pool,      # P tiles (diag)
            tc.tile_pool(name="on", bufs=int(os.environ.get("K_ON_BUFS", "4"))) as on_pool,      # [q,v] normalized
            tc.tile_pool(name="ot", bufs=int(os.environ.get("K_OT_BUFS", "4"))) as ot_pool,      # transposed [v,q]
            tc.tile_pool(name="rc", bufs=3) as rc_pool,      # reciprocals
            tc.tile_pool(name="yo", bufs=int(os.environ.get("K_YO_BUFS", "8"))) as yo_pool,      # y staging
            tc.tile_pool(name="ps_s", bufs=3, space="PSUM") as ps_s,
            tc.tile_pool(name="ps_ay", bufs=1, space="PSUM") as ps_ay,
            tc.tile_pool(name="ps_o", bufs=1, space="PSUM") as ps_o,
        ):
            xt_hi = pp.tile([128, 4, N], f8)
            xt_lo = pp.tile([128, 4, N], f8)
            wq_sb = pp.tile([128, 2, 4, 128], f8)
            wk_sb = pp.tile([128, 2, 4, 128], f8)
            wv_sb = pp.tile([128, 2, 4, 128], f8)
            wp_sb = pp.tile([128, C], bf)
            tri_sb = pp.tile([128, 128], bf)
            if Q_SINGLE:
                qT8s = pp.tile([128, N], f8)  # single fp8, dup via stride-0
            else:
                qT8 = pp.tile([128, 2, N], f8)   # (hi, lo) fp8 pair
            kT8 = pp.tile([128, N], f8)      # single fp8
            v_sb = pp.tile([128, NKB, 130], bf)

            # wq gates the very first QKV matmul: put it first on the fast
            # HWDGE (SP) queue, ahead of the x tiles; wk/wv on the ACT queue.
            # Later-needed constants (wp/tri/dmask) go via the slow Pool path.
            wqeng = nc.gpsimd if os.environ.get("K_WQG", "0") == "1" else nc.sync
            wqeng.dma_start(out=wq_sb[:, :, :, :], in_=wq8[:, :, :, :])
            if os.environ.get("K_XG", "0") == "1":
                # tile-0 x via the parallel SWDGE path, ahead of the weights:
                # HWDGE then only generates wq before the first matmul
                nc.gpsimd.dma_start(out=xt_hi[:, :, 0:QT], in_=xth[:, 0, :, :])
                nc.gpsimd.dma_start(out=xt_lo[:, :, 0:QT], in_=xtl[:, 0, :, :])
                globals()["_X0_DONE"] = True
            weng = nc.gpsimd if os.environ.get("K_WG", "1") == "1" else nc.scalar
            weng.dma_start(out=wk_sb[:, :, :, :], in_=wk8[:, :, :, :])
            weng.dma_start(out=wv_sb[:, :, :, :], in_=wv8[:, :, :, :])
            ceng = nc.gpsimd if os.environ.get("K_CG", "1") == "1" else nc.scalar
            ceng.dma_start(out=tri_sb, in_=tri[:, :])
            dmask_sb = pp.tile([128, 640], bf)
            ceng.dma_start(out=dmask_sb, in_=dmask[:, :])
            ceng.dma_start(out=wp_sb, in_=wp[:, :])
            # ones columns for the softmax denominator (cols 64 and 129)
            nc.vector.memset(
                v_sb.rearrange("p k (h j) -> p k h j", h=2)[:, :, :, 64], 1.0
            )
            # trigger the Exp act-table load early, overlapped with input DMAs
            warm = pp.tile([128, 1], f32)
            nc.vector.memset(warm, 0.0)
            nc.scalar.activation(warm, warm, Exp)


            UNSCALE = 1.0 / 64.0

            def kdup(b0, kb):
                return (
                    kT8[b0:b0 + 64, KB * kb:KB * (kb + 1)]
                    .unsqueeze(1)
                    .broadcast_to([64, 2, KB])
                )

            def qmov(b0, c0, c1):
                if Q_SINGLE:
                    # both DR slots read the same q8: result doubles, the
                    # 1/2 is folded into the exp scale
                    return (
                        qT8s[b0:b0 + 64, c0:c1]
                        .unsqueeze(1)
                        .broadcast_to([64, 2, c1 - c0])
                    )
                return qT8[b0:b0 + 64, :, c0:c1]

            # -------- QKV phase: fp8 DoubleRow, 3-term hi/lo compensation --
            def qkv_mms(ps, wsb, n, width=QT, dst_off=0):
                terms = [(0, xt_hi), (0, xt_lo), (1, xt_hi)]
                nmm = 0
                for s, xt8 in terms:
                    for cp in range(2):
                        nc.tensor.matmul(
                            ps[:, dst_off:dst_off + width],
                            wsb[:, s, 2 * cp:2 * cp + 2, :],
                            xt8[:, 2 * cp:2 * cp + 2, QT * n:QT * n + width],
                            start=(nmm == 0),
                            stop=(nmm == 5),
                            perf_mode=DR,
                        )
                        nmm += 1

            def pa_q(n, with_dma):
                def piece():
                    if with_dma and n == 0 and os.environ.get("K_XG", "0") == "1":
                        pass  # already issued in the startup block
                    elif with_dma and n == 0 and HEAD_SPLIT:
                        # fan the critical first tile across 2 HWDGE queues
                        nc.sync.dma_start(
                            out=xt_hi[:, :, 0:512], in_=xth[:, 0, :, :])
                        nc.scalar.dma_start(
                            out=xt_lo[:, :, 0:512], in_=xtl[:, 0, :, :])
                    elif with_dma:
                        nc.sync.dma_start(
                            out=xt_hi[:, :, QT * n:QT * (n + 1)],
                            in_=xth[:, n, :, :],
                        )
                        nc.sync.dma_start(
                            out=xt_lo[:, :, QT * n:QT * (n + 1)],
                            in_=xtl[:, n, :, :],
                        )
                    ps = ps_ay.tile([128, 512], f32, tag="ay", name=f"paq_{n}")
                    pq = ps[:, 0:512]
                    qkv_mms(pq, wq_sb, n)
                    if Q_SINGLE:
                        nc.scalar.activation(qT8s[:, QT * n:QT * (n + 1)], pq, Copy)
                        bal.add("act", A_COPY)
                    else:
                        # Pool can read PSUM: route the prompt-dependency
                        # evacuations there, freeing the exp engines
                        if POOL_PA:
                            nc.gpsimd.tensor_copy(
                                qT8[:, 0, QT * n:QT * (n + 1)], pq)
                        else:
                            nc.scalar.activation(qT8[:, 0, QT * n:QT * (n + 1)], pq, Copy)
                            bal.add("act", A_COPY)
                        qlo_eng = nc.gpsimd if (POOL_QLO or POOL_PA) else nc.vector
                        qlo_eng.tensor_tensor(
                            out=qT8[:, 1, QT * n:QT * (n + 1)],
                            in0=pq,
                            in1=qT8[:, 0, QT * n:QT * (n + 1)],
                            op=Alu.subtract,
                        )
                        if not (POOL_QLO or POOL_PA):
                            bal.add("dve", D_COPY)
                return piece

            def pa_k(n):
                def piece():
                    ps = ps_ay.tile([128, 512], f32, tag="ay", name=f"pak_{n}")
                    pk = ps[:, 0:512]
                    qkv_mms(pk, wk_sb, n)
                    if POOL_PA:
                        nc.gpsimd.tensor_copy(kT8[:, QT * n:QT * (n + 1)], pk)
                    else:
                        nc.scalar.activation(kT8[:, QT * n:QT * (n + 1)], pk, Copy)
                        bal.add("act", A_COPY)
                return piece

            def pa_v(n):
                # 4 kb blocks' V in one psum tile, one batched copy out
                def piece():
                    ps = ps_ay.tile([128, 512], f32, tag="ay", name=f"pav_{n}")
                    for j in range(4):
                        kb = 4 * n + j
                        pv = ps[:, 128 * j:128 * (j + 1)]
                        terms = [(xt_hi, 0), (xt_lo, 0), (xt_hi, 1)]
                        nmm = 0
                        for xt8, s in terms:
                            for cp in range(2):
                                nc.tensor.matmul(
                                    pv,
                                    xt8[:, 2 * cp:2 * cp + 2, KB * kb:KB * (kb + 1)],
                                    wv_sb[:, s, 2 * cp:2 * cp + 2, :],
                                    start=(nmm == 0),
                                    stop=(nmm == 5),
                                    perf_mode=DR,
                                )
                                nmm += 1
                    if POOL_PA:
                        nc.gpsimd.tensor_scalar(
                            out=v_sb[:, 4 * n:4 * n + 4, :]
                            .rearrange("p k (h j) -> p k h j", h=2)[:, :, :, 0:64],
                            in0=ps[:, 0:512].rearrange("p (k h j) -> p k h j", k=4, h=2),
                            scalar1=UNSCALE,
                            scalar2=None,
                            op0=Alu.mult,
                        )
                    else:
                        nc.scalar.activation(
                            v_sb[:, 4 * n:4 * n + 4, :]
                            .rearrange("p k (h j) -> p k h j", h=2)[:, :, :, 0:64],
                            ps[:, 0:512].rearrange("p (k h j) -> p k h j", k=4, h=2),
                            Copy,
                            scale=UNSCALE,
                        )
                        bal.add("act", A_COPY)
                return piece

            def phase_a_pieces(n):
                return [pa_q(n, True), pa_k(n), pa_v(n)]

            exp_idx = [0]

            def do_exp(dst, src, width, force=None):
                """Split exp between ACT (exact) and DVE (Schraudolph)."""
                sc = QK_SCALE * (0.5 if Q_SINGLE else 1.0)
                if force is None:
                    eng = bal.pick_exp(A_EXP[width], D_EXP[width], exp_idx[0], PATTERN)
                else:
                    eng = force
                    bal.add(force, D_EXP[width] if force == "dve" else A_EXP[width])
                exp_idx[0] += 1
                if eng == "act":
                    nc.scalar.activation(
                        dst[:, 0:width], src[:, 0:width], Exp, scale=sc
                    )
                else:
                    nc.vector.tensor_scalar(
                        out=dst[:, 0:width].bitcast(i16),
                        in0=src[:, 0:width],
                        scalar1=SCH_A * sc,
                        scalar2=SCH_B,
                        op0=Alu.mult,
                        op1=Alu.add,
                    )

            # diag slot layout keeps every matmul inside one 2KB PSUM bank:
            # r1 -> [0:384], r3 -> [384:512] (bank 0), r2 -> [512:768] (bank 1)
            offs = (0, 512, 384)
            wid = (384, 256, 128)

            psO_map = {}
            on_map = {}
            ot_map = {}
            import heapq
            deferred = []
            seq_counter = [0]

            def defer(due, fn):
                heapq.heappush(deferred, (due, seq_counter[0], fn))
                seq_counter[0] += 1

            def flush(i):
                while deferred and deferred[0][0] <= i:
                    heapq.heappop(deferred)[2]()

            def get_psO(qt, h):
                key = (qt, h)
                if key not in psO_map:
                    # interleaved streams: h1's accumulator lives in the
                    # QKV-phase bank (free once phase A is done)
                    if ILV and qt >= ILV_LO and h == 1:
                        psO_map[key] = ps_ay.tile([128, 4, 128], f32, tag="ay", name=f"psO_{qt}_{h}")
                    else:
                        psO_map[key] = ps_o.tile([128, 4, 128], f32, tag="o", name=f"psO_{qt}_{h}")
                return psO_map[key]

            def get_on(qt):
                if qt not in on_map:
                    on_map[qt] = on_pool.tile([128, 4, 128], bf, tag="on", name=f"on_{qt}")
                return on_map[qt]

            # --- AV' matmuls: P stationary [128 keys, 128 q], V moving [128,65]
            # PSUM has_written semantics: start=True clears the bits for the
            # WHOLE bank, so only the very first matmul of each (qt,h) stream
            # may set it. Later first-writes to other qc ranges overwrite
            # where the bit is unset, which is exactly what we need.
            av_started = {}

            def make_av(qt, h, contribs, stop_pred):
                # contribs: list of (kb, P_ap_slice_fn(qc) -> AP, qc_range)
                def av():
                    psO = get_psO(qt, h)
                    for kb, pap, qcs in contribs:
                        for qc in qcs:
                            first = not av_started.get((qt, h))
                            av_started[(qt, h)] = True
                            nc.tensor.matmul(
                                psO[:, qc, 0:65],
                                pap(qc),
                                v_sb[:, kb, 65 * h:65 * h + 65],
                                start=first,
                                stop=stop_pred(kb, qc),
                                skip_group_check=True,
                            )
                return av

            def make_epilogue(qt, h):
                def epi():
                    psO = psO_map.pop((qt, h))
                    rc = rc_pool.tile([128, 4], f32, tag="rc")
                    nc.vector.reciprocal(out=rc, in_=psO[:, :, 64])
                    bal.add("dve", 129.0)
                    on = get_on(qt)
                    # single normalize op: rc broadcast along the v dim
                    nc.vector.tensor_tensor(
                        out=on[:, :, 64 * h:64 * h + 64],
                        in0=psO[:, :, 0:64],
                        in1=rc.unsqueeze(2).broadcast_to([128, 4, 64]),
                        op=Alu.mult,
                    )
                    bal.add("dve", 392.0)
                    if h == 1:
                        ot = ot_pool.tile([128, 512], bf, tag="ot", name=f"ot_{qt}")
                        ot_map[qt] = ot
                        on_map.pop(qt)
                        for qc in range(4):
                            nc.sync.dma_start(
                                out=ot[:, 128 * qc:128 * (qc + 1)],
                                in_=on[:, qc, :],
                                transpose=True,
                            )
                return epi

            def make_proj_ob(qt, ob):
                def proj():
                    ot = ot_map[qt]
                    # the last q-tile's projections run in the drain; borrow
                    # the then-idle S pool for double buffering
                    borrow_s = qt == NQT - 1 or (ILV and ILV_BS and qt >= NQT - 3)
                    pool = ps_s if borrow_s else ps_ay
                    tag = "s" if borrow_s else "ay"
                    psY = pool.tile([128, 512], f32, tag=tag, name=f"psY_{qt}_{ob}")
                    nc.tensor.matmul(
                        psY,
                        wp_sb[:, 128 * ob:128 * (ob + 1)],
                        ot,
                        start=True,
                        stop=True,
                    )
                    y_sb = yo_pool.tile([128, 512], bf if YBF else f32, tag="yo")
                    if POOL_Y:
                        nc.gpsimd.tensor_copy(y_sb, psY)
                        if YDMA_POOL:
                            deng = nc.gpsimd
                        else:
                            deng = nc.sync if qt < NQT - 1 else (nc.sync, nc.scalar)[ob % 2]
                    else:
                        eng = bal.pick(A_COPY, D_COPY)
                        if eng == "act":
                            nc.scalar.activation(y_sb, psY, Copy)
                        else:
                            nc.vector.tensor_copy(y_sb, psY)
                        if Y_ALL_SYNC:
                            deng = nc.sync if qt < NQT - 1 else (nc.sync, nc.scalar)[ob % 2]
                        elif qt == NQT - 1:
                            if os.environ.get("K_YDR", "0") == "1":
                                deng = nc.gpsimd
                            else:
                                deng = (nc.sync, nc.gpsimd, nc.scalar, nc.gpsimd)[ob]
                        else:
                            deng = nc.sync if ob % 2 == 0 else nc.gpsimd
                    deng.dma_start(
                        out=yt[128 * ob:128 * (ob + 1), QT * qt:QT * (qt + 1)],
                        in_=y_sb,
                    )
                    if ob == 3:
                        ot_map.pop(qt)
                return proj

            ui = 0
            pa_pending = []

            def tick():
                nonlocal ui
                if TICK_MODE == 1:
                    if pa_pending:
                        pa_pending.pop(0)()
                    flush(ui)
                else:
                    if TICK_MODE != 2:
                        flush(ui)
                    if pa_pending:
                        pa_pending.pop(0)()
                ui += 1

            def emit_pair_unit(qt, h, kb, w, stop_pred, av_lag):
                b0 = 64 * h
                kbs = list(range(kb, kb + w))
                is_singleton = kbs[-1] == 4 * qt
                psS = ps_s.tile([128, 1024], f32, tag="s")
                for j, kbj in enumerate(kbs):
                    nc.tensor.matmul(
                        psS[:, 512 * j:512 * (j + 1)],
                        kdup(b0, kbj),
                        qmov(b0, QT * qt, QT * (qt + 1)),
                        start=True,
                        stop=True,
                        perf_mode=DR,
                    )
                if TICK_MODE == 2:
                    flush(ui)
                Pf = pf_pool.tile([128, 1024], bf, tag="pf")
                do_exp(Pf, psS, 512 * w,
                       force="dve" if (is_singleton and MASK_DVE) else None)
                if is_singleton:
                    j = w - 1
                    meng = nc.vector if MASK_DVE else nc.gpsimd
                    meng.tensor_mul(
                        Pf[:, 512 * j:512 * j + 128],
                        Pf[:, 512 * j:512 * j + 128],
                        tri_sb,
                    )
                    if MASK_DVE:
                        bal.add("dve", 127.0)
                # AV contributions of this unit (bind Pf NOW — late
                # binding would capture a future unit's tile)
                contribs = [
                    (kbj, (lambda jj, P: lambda qc: P[:, 512 * jj + 128 * qc:512 * jj + 128 * qc + 128])(j2, Pf), range(4))
                    for j2, kbj in enumerate(kbs)
                ]
                defer(ui + av_lag, make_av(qt, h, contribs, stop_pred))
                tick()

            def emit_diag_unit(qt, h, stop_pred, av_lag, mask_on_dve):
                b0 = 64 * h
                psD = ps_s.tile([128, 1024], f32, tag="s")
                for r in (1, 2, 3):
                    kbr = 4 * qt + r
                    nc.tensor.matmul(
                        psD[:, offs[r - 1]:offs[r - 1] + wid[r - 1]],
                        kdup(b0, kbr),
                        qmov(b0, QT * qt + 128 * r, QT * qt + 128 * r + wid[r - 1]),
                        start=True,
                        stop=True,
                        perf_mode=DR,
                    )
                if TICK_MODE == 2:
                    flush(ui)
                Pd = pd_pool.tile([128, 768], bf, tag="pd")
                do_exp(Pd, psD, 768, force="dve" if MASK_DVE else None)
                if mask_on_dve or MASK_DVE:
                    nc.vector.tensor_mul(Pd[:, 0:640], Pd[:, 0:640], dmask_sb)
                    bal.add("dve", 393.0)
                else:
                    nc.gpsimd.tensor_mul(Pd[:, 0:640], Pd[:, 0:640], dmask_sb)
                contribs = [
                    (4 * qt + r,
                     (lambda rr, P: lambda qc: P[:, offs[rr - 1] + 128 * (qc - rr):offs[rr - 1] + 128 * (qc - rr) + 128])(r, Pd),
                     range(r, 4))
                    for r in (1, 2, 3)
                ]
                defer(ui + av_lag, make_av(qt, h, contribs, stop_pred))
                tick()

            for n in (0, 1):
                for piece in phase_a_pieces(n):
                    piece()
            for qt in range(NQT):
                for piece in pa_pending:
                    piece()
                if ILV and ILV5:
                    # free the ay slot from qt5 on: tiles 6 AND 7 prefetch
                    # during qt4
                    if qt + 2 < NQT - 1:
                        pa_pending = phase_a_pieces(qt + 2)
                        if qt + 2 == NQT - 2:
                            pa_pending = pa_pending + phase_a_pieces(NQT - 1)
                    else:
                        pa_pending = []
                else:
                    pa_pending = phase_a_pieces(qt + 2) if qt + 2 < NQT else []
                if ILV and qt >= ILV_LO:
                    # interleave the two head-streams: double the independent
                    # work visible to the strictly in-order engine queues
                    assert EPI_LAG >= AVD_LAG and EPI_LAG >= AVP_LAG - 1
                    def units_for(_q):
                        us = []
                        kb = 0
                        while kb <= 4 * _q:
                            w = min(2, 4 * _q + 1 - kb)
                            us.append(("pair", kb, w))
                            kb += w
                        us.append(("diag",))
                        return us
                    u0, u1 = units_for(qt), units_for(qt)
                    sp = lambda kb, qc, _q=qt: kb == 4 * _q + qc
                    # drain stream: pull its diag forward so the epilogue
                    # hangs off the singleton pair unit (stop on kb==4qt)
                    de = (qt == NQT - 1
                          and os.environ.get("K_DE", "0") == "1")
                    if de:
                        u1.insert(len(u1) - 4, u1.pop())
                    sp1 = (lambda kb, qc, _q=qt: kb == 4 * _q) if de else sp
                    i = j = t = 0
                    while i < len(u0) or j < len(u1):
                        if i < len(u0) and (t < ILV_OFS or j >= len(u1)
                                            or (t - ILV_OFS) % 2 == 0):
                            hh, unit = 0, u0[i]
                            i += 1
                            is_last = i == len(u0)
                        else:
                            hh, unit = 1, u1[j]
                            j += 1
                            is_last = j == len(u1)
                        sph = sp1 if hh == 1 else sp
                        if unit[0] == "pair":
                            emit_pair_unit(qt, hh, unit[1], unit[2], sph, AVP_LAG)
                        else:
                            emit_diag_unit(qt, hh, sph, AVD_LAG,
                                           hh == 1 and qt == NQT - 1)
                        if is_last:
                            # epilogue strictly after this stream's last AV
                            # (equal dues keep insertion order: AV first)
                            epi_lag = AVP_LAG if unit[0] == "pair" else EPI_LAG
                            defer(ui - 1 + epi_lag, make_epilogue(qt, hh))
                            if hh == 1:
                                lag0 = epi_lag + 1 if qt == NQT - 1 else PROJ_LAG
                                step = 1 if qt == NQT - 1 else 2
                                for ob in range(4):
                                    defer(ui - 1 + lag0 + step * ob,
                                          make_proj_ob(qt, ob))
                        t += 1
                    continue
                for h in range(2):
                    last_stream = qt == NQT - 1 and h == 1 and TAIL_REORDER
                    if last_stream:
                        # drain stream: diag first so the tail chain ends on a
                        # plain pair unit, masks on DVE, tight deferrals
                        stop_pred = lambda kb, qc, _q=qt: kb == 4 * _q
                        emit_diag_unit(qt, h, stop_pred, TAVP_LAG, True)
                        kb = 0
                        while kb <= 4 * qt:
                            w = min(2, 4 * qt + 1 - kb)
                            emit_pair_unit(qt, h, kb, w, stop_pred, TAVP_LAG)
                            kb += w
                        # epilogue strictly after the last unit's AV
                        # (due ui-1+TAVP_LAG); equal dues keep insertion order
                        defer(ui - 1 + TAVP_LAG, make_epilogue(qt, h))
                        for ob in range(4):
                            defer(ui + TAVP_LAG + ob, make_proj_ob(qt, ob))
                    else:
                        stop_pred = lambda kb, qc, _q=qt: kb == 4 * _q + qc
                        kb = 0
                        while kb <= 4 * qt:
                            w = min(2, 4 * qt + 1 - kb)
                            emit_pair_unit(qt, h, kb, w, stop_pred, AVP_LAG)
                            kb += w
                        # diag AV + epilogue land together at +3
                        emit_diag_unit(qt, h, stop_pred, AVD_LAG, False)
                        defer(ui + EPI_LAG, make_epilogue(qt, h))
                        if h == 1:
                            for ob in range(4):
                                defer(ui + PROJ_LAG + 2 * ob, make_proj_ob(qt, ob))
            flush(10 ** 9)

    nc.compile()
    if os.environ.get("KERNEL_BALANCE"):
        print(f"balancer: act={bal.act/1000:.1f}us dve={bal.dve/1000:.1f}us")
    return nc


def kernel(x, w_qkv, w_proj, b_proj):
    global LAST_RESULTS
    from concourse.bass_utils import run_bass_kernel_spmd

    if "nc" not in _CACHE:
        _CACHE["nc"] = _build()
    nc = _CACHE["nc"]

    x = np.asarray(x)
    w_qkv = np.asarray(w_qkv)
    w_proj = np.asarray(w_proj)
    b_proj = np.asarray(b_proj)
    bf16 = ml_dtypes.bfloat16
    e4m3 = ml_dtypes.float8_e4m3fn
    scale = D ** -0.5

    def split8(a):
        hi = a.astype(e4m3)
        lo = (a - hi.astype(np.float32)).astype(e4m3)
        # [2, C, 128] -> [p, s, c_chunk, f]: contiguous 1KB partition lines
        return np.ascontiguousarray(
            np.stack([hi, lo]).reshape(2, 4, 128, 128).transpose(2, 0, 1, 3))

    tri = np.triu(np.ones((128, 128), np.float32)).astype(bf16)
    trif = np.triu(np.ones((128, 128), np.float32))
    dm = np.ones((128, 640), np.float32)
    dm[:, 0:128] = trif
    dm[:, 384:512] = trif
    dm[:, 512:640] = trif
    dm = dm.astype(bf16)
    in_maps = []
    for core in range(8):
        b, g = divmod(core, 4)
        xt32 = np.ascontiguousarray(x[b].T).astype(np.float32)
        xt_hi = xt32.astype(e4m3)
        xt_lo = (xt32 - xt_hi.astype(np.float32)).astype(e4m3)
        # retile [C, N] -> [p, n_tile, c_chunk, col] so each DMA descriptor
        # covers a full 2KB partition line
        xt_hi = np.ascontiguousarray(
            xt_hi.reshape(4, 128, NQT, QT).transpose(1, 2, 0, 3))
        xt_lo = np.ascontiguousarray(
            xt_lo.reshape(4, 128, NQT, QT).transpose(1, 2, 0, 3))
        wq_ = split8(np.ascontiguousarray(w_qkv[128 * g:128 * (g + 1), :].T * (scale * 64.0)))
        wk_ = split8(np.ascontiguousarray(w_qkv[C + 128 * g:C + 128 * (g + 1), :].T) * 64.0)
        wv_ = split8(np.ascontiguousarray(w_qkv[2 * C + 128 * g:2 * C + 128 * (g + 1), :].T) * 64.0)
        wp_ = np.ascontiguousarray(w_proj[:, 128 * g:128 * (g + 1)].T).astype(bf16)
        in_maps.append({
            "xth": xt_hi, "xtl": xt_lo,
            "wq8": wq_, "wk8": wk_, "wv8": wv_,
            "wp": wp_, "tri": tri, "dmask": dm,
        })

    res = run_bass_kernel_spmd(
        nc,
        in_maps,
        core_ids=list(range(8)),
        trace=bool(os.environ.get("KERNEL_TRACE")),
    )
    LAST_RESULTS = res

    y = np.empty((B, N, C), np.float32)
    for b in range(B):
        acc = res.results[4 * b]["yt"].astype(np.float32)
        for g in range(1, 4):
            acc = acc + res.results[4 * b + g]["yt"]
        y[b] = acc.T + b_proj
    return y
